# revision 5
# baseline (speedup 1.0000x reference)
import os

os.environ.setdefault("NEURON_CC_FLAGS", "--auto-cast=none")

import numpy as np
import jax
import jax.numpy as jnp

GROUPS = 8
GP = 64
K = 64
C_IN = 512
EPS = 1e-5
N_CORES = 8
F32 = jnp.float32
BF16 = jnp.bfloat16


def _fwd_impl(xn, qkv_wT, g1, b1, g2, g3, b3, weight, gamma, emb_q, emb_k, emb_v):
    """Per-device axial attention. xn: [C,H,W] fp32 for batch element n.

    bf16 matmuls with fp32 accumulation; three fused cross-device stat
    allreduces (one per BN). BN2's mean/bias terms are per-(comp,g) scalars,
    constant along the softmax axis, so only the BN2 scales are applied.
    BN3 is folded into the final residual as a per-channel affine.
    """
    C, H, W = xn.shape
    xb = xn.astype(BF16)

    # ---- qkv conv: [O,C]@[C,H*W] -> [O,H,W] ----
    qkv = jnp.einsum("co,chw->ohw", qkv_wT, xb,
                     preferred_element_type=F32).astype(BF16)  # [1024,H,W]

    # BN1 stats (global over all devices' (h,w) samples)
    qf = qkv.astype(F32)
    s1 = qf.mean((1, 2))
    s2 = jnp.square(qf).mean((1, 2))
    st = jax.lax.pmean(jnp.stack([s1, s2]), "i")            # AR #1: [2,1024]
    m1 = st[0]
    v1 = st[1] - m1 * m1
    sc1 = jax.lax.rsqrt(v1 + EPS) * g1
    bi1 = b1 - m1 * sc1
    qn = (qf * sc1[:, None, None] + bi1[:, None, None]).astype(BF16)

    qg = qn.reshape(GROUPS, 2 * GP, H, W)
    q = qg[:, : GP // 2]                                    # [G,32,H,W]
    k = qg[:, GP // 2: GP]                                  # [G,32,H,W]
    v = qg[:, GP:]                                          # [G,64,H,W]

    # ---- attention logits (w is the batch axis) ----
    # bf16 materialization: halves HBM traffic for the three [G,W,64,64]
    # tensors; stats are still accumulated in f32 below.
    qk = jnp.einsum("gciw,gcjw->gwij", q, k,
                    preferred_element_type=F32).astype(BF16)
    qr = jnp.einsum("gciw,cij->gwij", q, emb_q,
                    preferred_element_type=F32).astype(BF16)
    kr = jnp.einsum("gcjw,cji->gwij", k, emb_k,
                    preferred_element_type=F32).astype(BF16)

    # BN2: softmax removes per-(g) mean/bias terms; only scales matter.
    def _ms(t):
        tf = t.astype(F32)
        return tf.mean((1, 2, 3)), jnp.square(tf).mean((1, 2, 3))

    st2 = jnp.stack([*_ms(qk), *_ms(qr), *_ms(kr)])         # [6,G]
    st2 = jax.lax.pmean(st2, "i")                           # AR #2: [6,8]
    s_qk = jax.lax.rsqrt(st2[1] - st2[0] ** 2 + EPS) * g2[0]
    s_qr = jax.lax.rsqrt(st2[3] - st2[2] ** 2 + EPS) * g2[1]
    s_kr = jax.lax.rsqrt(st2[5] - st2[4] ** 2 + EPS) * g2[2]

    logits = (qk.astype(F32) * s_qk[:, None, None, None]
              + qr.astype(F32) * s_qr[:, None, None, None]
              + kr.astype(F32) * s_kr[:, None, None, None])
    sim = jax.nn.softmax(logits, axis=-1).astype(BF16)      # [G,W,i,j]

    # ---- output projection ----
    wb = weight.astype(BF16)                                # [64,64]
    sw = jnp.einsum("gwij,io->gwjo", sim, wb,
                    preferred_element_type=F32).astype(BF16)
    sv = jnp.einsum("gcjw,gwjo->gcow", v, sw,
                    preferred_element_type=F32).astype(BF16)   # [G,64,o,W]
    s1e = jnp.einsum("gwij,cij->gwci", sim, emb_v,
                     preferred_element_type=F32).astype(BF16)
    sve = jnp.einsum("gwci,io->gcow", s1e, wb,
                     preferred_element_type=F32).astype(BF16)  # [G,64,o,W]

    # BN3 stats per channel: ch = g*128 + 2c (sv) / +1 (sve); stats over (w,o)
    svf = sv.astype(F32)
    svef = sve.astype(F32)
    s3a = jnp.stack([svf.mean((2, 3)), jnp.square(svf).mean((2, 3)),
                     svef.mean((2, 3)), jnp.square(svef).mean((2, 3))])  # [4,G,64]
    s3a = jax.lax.pmean(s3a, "i")                           # AR #3: [4,8,64]
    ge = g3.reshape(GROUPS, GP, 2)
    be = b3.reshape(GROUPS, GP, 2)
    sc_e = jax.lax.rsqrt(s3a[1] - s3a[0] ** 2 + EPS) * ge[..., 0]
    sc_o = jax.lax.rsqrt(s3a[3] - s3a[2] ** 2 + EPS) * ge[..., 1]
    cb = (be[..., 0] - s3a[0] * sc_e) + (be[..., 1] - s3a[2] * sc_o)

    out = (svf * sc_e[:, :, None, None] + svef * sc_o[:, :, None, None]
           + cb[:, :, None, None])                          # [G,64,o=H,W]
    return xn + gamma * out.reshape(C, H, W)


_fwd = jax.pmap(
    _fwd_impl, axis_name="i",
    in_axes=(0,) + (None,) * 11)

_fwd_all0 = jax.pmap(_fwd_impl, axis_name="i")


def _host_prep(qkv_w, relative, pos_map, weight):
    rel_idx = np.arange(K)[:, None] - np.arange(K)[None, :] + K - 1
    all_emb = np.asarray(relative)[:, rel_idx] + np.asarray(pos_map)  # [128,K,K]
    emb_q = all_emb[: GP // 2].astype(BF16)
    emb_k = all_emb[GP // 2: GP].astype(BF16)
    emb_v = all_emb[GP:].astype(BF16)
    qkv_wT = np.ascontiguousarray(np.asarray(qkv_w).T).astype(BF16)   # [C,O]
    return qkv_wT, emb_q, emb_k, emb_v


def kernel(x, qkv_w, bn_qkv_g, bn_qkv_b, bn_sim_g, bn_sim_b, bn_out_g, bn_out_b,
           weight, relative, gamma, pos_map):
    x = np.asarray(x, np.float32)
    qkv_wT, emb_q, emb_k, emb_v = _host_prep(qkv_w, relative, pos_map, weight)
    g2 = np.asarray(bn_sim_g, np.float32).reshape(3, GROUPS)
    out = _fwd(x, qkv_wT,
               np.asarray(bn_qkv_g, np.float32), np.asarray(bn_qkv_b, np.float32),
               g2,
               np.asarray(bn_out_g, np.float32), np.asarray(bn_out_b, np.float32),
               np.asarray(weight, np.float32),
               np.float32(gamma), emb_q, emb_k, emb_v)
    return np.asarray(out, np.float32)
